# revision 23
# baseline (speedup 1.0000x reference)
"""Trainium2 Bass kernel for nn_MinRNNPredictor (2-layer minGRU + FC head).

Sharding: data-parallel over batch — each of the 8 NeuronCores runs the
full network on one batch row (the recurrence is independent per row);
the small weight matrices are replicated. No collectives.

Per-core dataflow (all on-chip tensors in [feature, time] layout):
  x.T (bf16, cast + pre-transposed on host; plain contiguous DMA loads)
    -> GEMM0 (PE, bf16 in / fp32 PSUM): pre_z0, pre_h0  [H, Tc]
    -> gates (ScalarE sigmoid, DVE scalar_tensor_tensor)
    -> h0 via DVE TensorTensorScan along the free/time axis
       (+ ScalarE cast of h0 to fp8 for the next layer's z-GEMM)
    -> GEMM1: z-gate GEMM in fp8e4 *DoubleRow* perf mode (2 k-subtiles
       per matmul, ~1.8x PE throughput; the gate tolerates fp8 since the
       sigmoid + scan attenuate the quantization noise), candidate GEMM
       in bf16 -> gates -> scan -> h1
    -> FC with h1 stationary, j-outer accumulation (starts as soon as
       the first h1 subtile lands — shrinks the pipeline-drain tail),
       bias added via a K=1 matmul, PSUM evacuated by ScalarE.

Weights and x are cast to bf16 (Wz1: fp8, x16 scale) on the host once;
biases are pre-striped/broadcast on host so each is one clean DMA.

The time axis is processed in chunks of 512 (one PSUM bank). The chunk
loop is software-pipelined: per iteration the PE runs GEMM0(i),
GEMM1(i-1) and FC(i-2), so the serial DVE scan chain of a chunk always
overlaps a full iteration of PE work instead of stalling the PE at
chunk boundaries. Chunk 0 of layer 0 runs k-outer (all m-tiles per
k-slice) so the cold-start GEMM only ever waits on the *next* 256 KB
weight slice instead of the full Wz0+Wh0.
"""

import os

# This kernel must run on the axon-tunneled NeuronCores. A host process may
# pin JAX_PLATFORMS=cpu for its own reference math; drop such a pin before
# jax is imported (via concourse) so jax.devices() still sees the cores.
_jp = os.environ.get("JAX_PLATFORMS")
if _jp is not None and "axon" not in _jp and "neuron" not in _jp:
    os.environ.pop("JAX_PLATFORMS", None)

from contextlib import ExitStack

import ml_dtypes
import numpy as np

import concourse.mybir as mybir
import concourse.tile as tile
from concourse import bacc, bass_utils

P = 128
B, T, DIN, H, DOUT = 8, 4096, 512, 1024, 512
TC = 512  # time-chunk = one PSUM bank of fp32

F32 = mybir.dt.float32
BF16 = mybir.dt.bfloat16
FP8 = mybir.dt.float8e4
ALU = mybir.AluOpType
ACTF = mybir.ActivationFunctionType
DR = mybir.MatmulPerfMode.DoubleRow

# fp8 staging scales: Wz1 is stored as fp8(16*Wz1), h0 as fp8(4*h0); the
# sigmoid activations undo the product scale with scale=1/64.
WZ1_SCALE = 16.0
H0_SCALE = 4.0
Z1_DESCALE = 1.0 / (WZ1_SCALE * H0_SCALE)


def build(t_total=T, tcc=TC):
    """Build the single-core Bass module (same NEFF runs SPMD on all cores)."""
    nchunk = t_total // tcc
    assert t_total % tcc == 0 and tcc % P == 0
    hsub = H // P
    ksub0 = DIN // P

    nc = bacc.Bacc("TRN2", target_bir_lowering=False, debug=False, num_devices=B)
    # x pre-transposed on host to [DIN/P, P, T] so every [P, tcc] x.T tile
    # is a single clean contiguous-row DMA (no xbar transposes on chip).
    x_d = nc.dram_tensor("xT", [DIN // P, P, t_total], BF16, kind="ExternalInput").ap()
    w_d = {}
    for name, shape, dt in (
        ("Wz0", [DIN, H], BF16),
        ("Wh0", [DIN, H], BF16),
        ("Wz1q", [H, H], FP8),
        ("Wh1", [H, H], BF16),
        ("Wfc", [H, DOUT], BF16),
        # Biases pre-striped on host: [P, 6*hsub] columns are
        # [bz0, bh0, bz1, bh1, -bz0, -bz1] stripes of [P, hsub] each.
        # [bz0, bh0, bz1, bh1, -bz0, -bz1] stripes of [P, hsub] each,
        # then the bfc stripe [P, DOUT/P].
        ("bias_pack", [P, 6 * (H // P) + DOUT // P], F32),
    ):
        w_d[name] = nc.dram_tensor(name, shape, dt, kind="ExternalInput").ap()
    # y in [DOUT, T] layout: the FC runs with Wfc stationary so its PSUM
    # output has Dout on partitions, which lets ScalarE's per-partition
    # activation bias apply bfc during PSUM evacuation (no bias matmuls).
    # The host transposes back to [T, DOUT].
    y_d = nc.dram_tensor("y", [DOUT, t_total], F32, kind="ExternalOutput").ap()

    with tile.TileContext(nc) as tc, ExitStack() as ctx:
        const = ctx.enter_context(tc.tile_pool(name="const", bufs=1))
        sb = ctx.enter_context(tc.tile_pool(name="sb", bufs=2))
        psum = ctx.enter_context(tc.tile_pool(name="psum", bufs=8, space="PSUM"))

        xT_tiles = {}
        h0_tiles = {}
        h0q_tiles = {}
        h1_tiles = {}
        carry0 = [None] * hsub
        carry1 = [None] * hsub

        def emit_T(i):
            """Load the x.T tiles of chunk i (host pre-transposed)."""
            xT = []
            for dj in range(DIN // P):
                t_ = sb.tile([P, tcc], BF16, tag=f"xT{dj}", bufs=3, name=f"xT{dj}_{i}")
                nc.gpsimd.dma_start(t_[:], x_d[dj, :, i * tcc : (i + 1) * tcc])
                xT.append(t_)
            xT_tiles[i] = xT

        # PE warmup: the HAM clock gate holds the PE at half clock until it
        # has seen ~3.4us of sustained activity. The PE is idle waiting on
        # the first weight DMAs at kernel start anyway, so burn that window
        # on zero matmuls to arrive at the first real GEMM near full clock.
        warm = const.tile([P, P], BF16, name="warm")
        nc.vector.memset(warm[:], 0.0)
        wp = psum.tile([P, P], F32, tag="psum", name="warm_psum")
        # 32 burns = exactly one 3.4us HAM window: bridges the ~9us
        # DMA-engine startup dead-zone with dense PE activity so HAM flips
        # to full clock, without the burn queue delaying the first real GEMM.
        for _ in range(32):
            nc.tensor.matmul(wp[:], lhsT=warm[:], rhs=warm[:])

        # x chunk 0 first on the gpsimd DMA queue: it gates the first GEMM.
        emit_T(0)

        # Bias pack next: one tiny clean DMA, needed by the first gates.
        bias_sb = const.tile([P, 6 * hsub + DOUT // P], F32, name="bias_sb")
        nc.gpsimd.dma_start(bias_sb[:], w_d["bias_pack"])
        bz0_sb = bias_sb[:, 0 * hsub : 1 * hsub]
        bh0_sb = bias_sb[:, 1 * hsub : 2 * hsub]
        bz1_sb = bias_sb[:, 2 * hsub : 3 * hsub]
        bh1_sb = bias_sb[:, 3 * hsub : 4 * hsub]
        nbz0_sb = bias_sb[:, 4 * hsub : 5 * hsub]
        nbz1_sb = bias_sb[:, 5 * hsub : 6 * hsub]
        bfc_sb = bias_sb[:, 6 * hsub : 6 * hsub + DOUT // P]

        # Resident weights (HWDGE), contraction dim on partitions.
        # Layer-0 weights first — GEMM0(0) needs them earliest; the rest
        # stream in under the first chunks' compute.
        def load_w(name, k_dim, n_dim, dt=BF16, split=False, col_split=False):
            t_ = const.tile([P, k_dim // P, n_dim], dt, name=f"{name}_sb")
            src = w_d[name].rearrange("(o p) n -> p o n", p=P)
            if col_split:
                # Column-half x k-slice DMAs in chunk-0 consumption order
                # (k-outer over m-halves): the very first GEMM only waits on
                # a 128 KB quarter-slice instead of a full k-slice.
                h2 = n_dim // 2
                for c in range(2):
                    for k in range(k_dim // P):
                        nc.sync.dma_start(
                            t_[:, k : k + 1, c * h2 : (c + 1) * h2],
                            src[:, k : k + 1, c * h2 : (c + 1) * h2],
                        )
            elif split:
                # Per-k-tile DMAs: the first accumulation matmuls only wait
                # for their own k-slice instead of the whole weight.
                for k in range(k_dim // P):
                    nc.sync.dma_start(t_[:, k : k + 1, :], src[:, k : k + 1, :])
            else:
                nc.sync.dma_start(t_[:], src)
            return t_

        wz0_sb = load_w("Wz0", DIN, H, col_split=True)
        wh0_sb = load_w("Wh0", DIN, H, col_split=True)
        wz1_sb = load_w("Wz1q", H, H, dt=FP8, split=True)
        wh1_sb = load_w("Wh1", H, H, split=True)
        wfc_sb = load_w("Wfc", H, DOUT)

        def gates_scan(i, m, pz, ph, bz, nbz, bh, zscale, carry, ltag, cast_out):
            """sigmoids (ScalarE) -> b (DVE stt) -> scan (DVE) for one m-tile;
            optionally cast h to fp8*H0_SCALE (ScalarE) for the next layer."""
            # a = 1 - z = sigmoid(-(pre_z*s + bz)); z = sigmoid(pre_z*s + bz)
            a_t = sb.tile(
                [P, tcc], BF16, tag=f"a{ltag}", bufs=4, name=f"a{ltag}_{i}_{m}"
            )
            nc.scalar.activation(
                a_t[:], pz[:], ACTF.Sigmoid, bias=nbz[:, m : m + 1], scale=-zscale
            )
            z_t = sb.tile(
                [P, tcc], BF16, tag=f"z{ltag}", bufs=4, name=f"z{ltag}_{i}_{m}"
            )
            nc.scalar.activation(
                z_t[:], pz[:], ACTF.Sigmoid, bias=bz[:, m : m + 1], scale=zscale
            )
            # b = (pre_h + bh) * z
            b_t = sb.tile(
                [P, tcc], BF16, tag=f"b{ltag}", bufs=4, name=f"b{ltag}_{i}_{m}"
            )
            nc.vector.scalar_tensor_tensor(
                b_t[:], ph[:], bh[:, m : m + 1], z_t[:], op0=ALU.add, op1=ALU.mult
            )
            # h_t = a_t * h_{t-1} + b_t along the time (free) axis
            h_t = sb.tile(
                [P, tcc], BF16, tag=f"h{ltag}_{m}", bufs=3, name=f"h{ltag}_{i}_{m}"
            )
            init = 0.0 if carry[m] is None else carry[m][:, tcc - 1 : tcc]
            nc.vector.tensor_tensor_scan(
                h_t[:], a_t[:], b_t[:], init, op0=ALU.mult, op1=ALU.add
            )
            carry[m] = h_t
            if cast_out is not None:
                # fp8 cast on ScalarE. (GpSimd measured 7.6us per tile for
                # this — 11x ScalarE — and its SBUF port contention also
                # slowed DVE; keep it off GpSimd.)
                nc.scalar.activation(
                    cast_out[:, m, :], h_t[:], ACTF.Copy, scale=H0_SCALE
                )
            return h_t

        def layer0_steps(i):
            """x[P*ksub0, tcc] -> h0[i]: bf16 GEMMs + gates + scan; also
            produces the fp8 copy of h0 used by GEMM1's z-gate. Generator:
            yields after each m-tile so the chunk loop can interleave with
            layer 1 (smooths PSUM-ring pressure against the gate chain).
            Chunks 0-1 run k-outer so the cold-start GEMM only ever waits
            on the next 256 KB weight slice instead of all of Wz0+Wh0."""
            rhs = xT_tiles.pop(i)
            h0q = sb.tile([P, hsub, tcc], FP8, tag="h0q", bufs=3, name=f"h0q_{i}")
            h0q_tiles[i] = h0q
            outs = []
            h0_tiles[i] = outs
            k_outer = i <= 1

            def zh_mms(ms):
                pzs, phs = {}, {}
                for w, ps in ((wz0_sb, pzs), (wh0_sb, phs)):
                    for m in ms:
                        ps[m] = psum.tile(
                            [P, tcc], F32, tag="psum", name=f"p{i}_{m}_{id(ps)}"
                        )
                    order = (
                        [(k, m) for k in range(ksub0) for m in ms]
                        if k_outer
                        else [(k, m) for m in ms for k in range(ksub0)]
                    )
                    for k, m in order:
                        nc.tensor.matmul(
                            ps[m][:],
                            lhsT=w[:, k, m * P : (m + 1) * P],
                            rhs=rhs[k][:],
                            start=(k == 0),
                            stop=(k == ksub0 - 1),
                        )
                return pzs, phs

            groups = (
                [list(range(4 * mh, 4 * mh + 4)) for mh in (0, 1)]
                if k_outer
                else [[m] for m in range(hsub)]
            )
            for ms in groups:
                pzs, phs = zh_mms(ms)
                for m in ms:
                    outs.append(
                        gates_scan(
                            i, m, pzs[m], phs[m], bz0_sb, nbz0_sb, bh0_sb,
                            1.0, carry0, "0", h0q,
                        )
                    )
                    yield

        def layer1_steps(i):
            """h0[i] -> h1[i]: fp8 DoubleRow z-GEMM + bf16 candidate GEMM.
            Generator (yields per m-tile), same interleaving contract as
            layer0_steps. Chunk 0 runs k-outer so the cold-start GEMM only
            waits on the next Wh1 k-slice instead of the whole 2 MB weight."""
            rhs = h0_tiles.pop(i)
            h0q = h0q_tiles.pop(i)
            outs = []
            h1_tiles[i] = outs

            def z_mm(m, j):
                # z-gate GEMM: fp8 DoubleRow, two 128-row k-subtiles per
                # matmul (the PE packs 2 fp8 weights per cell).
                nc.tensor.matmul(
                    pzs[m][:],
                    lhsT=wz1_sb[:, 2 * j : 2 * j + 2, m * P : (m + 1) * P],
                    rhs=h0q[:, 2 * j : 2 * j + 2, :],
                    start=(j == 0),
                    stop=(j == hsub // 2 - 1),
                    perf_mode=DR,
                )

            def h_mm(m, k):
                nc.tensor.matmul(
                    phs[m][:],
                    lhsT=wh1_sb[:, k, m * P : (m + 1) * P],
                    rhs=rhs[k][:],
                    start=(k == 0),
                    stop=(k == hsub - 1),
                )

            def zh_mms(ms, k_outer):
                for m in ms:
                    pzs[m] = psum.tile([P, tcc], F32, tag="psum", name=f"pz1_{i}_{m}")
                    phs[m] = psum.tile([P, tcc], F32, tag="psum", name=f"ph1_{i}_{m}")
                if k_outer:
                    for j in range(hsub // 2):
                        for m in ms:
                            z_mm(m, j)
                    for k in range(hsub):
                        for m in ms:
                            h_mm(m, k)
                else:
                    # Alternate DoubleRow z-matmuls with bf16 h-matmuls: DR
                    # LDWEIGHTS cannot hide behind a back-to-back DR matmul
                    # (no background-buffer pull-ahead in DoubleRow mode), so
                    # give each DR weight load a full bf16 stream to load
                    # under. Both PSUM banks accumulate independently.
                    for m in ms:
                        for j in range(hsub // 2):
                            z_mm(m, j)
                            h_mm(m, 2 * j)
                            h_mm(m, 2 * j + 1)

            pzs, phs = {}, {}
            groups = [list(range(4 * mh, 4 * mh + 4)) for mh in (0, 1)] if i == 0 \
                else [[m] for m in range(hsub)]
            for ms in groups:
                zh_mms(ms, k_outer=(i == 0))
                for m in ms:
                    outs.append(
                        gates_scan(
                            i, m, pzs[m], phs[m], bz1_sb, nbz1_sb, bh1_sb,
                            Z1_DESCALE, carry1, "1", None,
                        )
                    )
                    yield

        def emit_FC(i):
            """y[i] = Wfc.T @ h1[i] (+ bfc), in [Dout, time] PSUM layout.
            j-outer accumulation: the first matmuls only need h1[i][0], so
            the tail chunk's FC overlaps the gate/scan chain instead of
            waiting for all of h1. The last k-slice runs per output block,
            immediately followed by that block's evacuation + DMA, so the
            drain pipelines instead of serializing after the last matmul.
            bfc rides in as ScalarE's per-partition activation bias."""
            h1 = h1_tiles.pop(i)
            no = DOUT // P
            last = i == nchunk - 1
            yps = [
                psum.tile([P, tcc], F32, tag="psum", name=f"yp_{i}_{o}")
                for o in range(no)
            ]
            for j in range(hsub - 1):
                for o in range(no):
                    nc.tensor.matmul(
                        yps[o][:],
                        lhsT=wfc_sb[:, j, o * P : (o + 1) * P],
                        rhs=h1[j][:],
                        start=(j == 0),
                        stop=False,
                    )
            for o in range(no):
                nc.tensor.matmul(
                    yps[o][:],
                    lhsT=wfc_sb[:, hsub - 1, o * P : (o + 1) * P],
                    rhs=h1[hsub - 1][:],
                    start=False,
                    stop=True,
                )
                y_sb = sb.tile([P, tcc], F32, tag="y", bufs=4, name=f"y_{i}_{o}")
                if last and o % 2 == 1:
                    # Tail chunk: split evacuation across ScalarE + DVE so the
                    # final drain runs on two engines in parallel.
                    nc.vector.tensor_scalar_add(
                        y_sb[:], yps[o][:], bfc_sb[:, o : o + 1]
                    )
                else:
                    # Identity (not Copy): Copy rejects AP biases.
                    nc.scalar.activation(
                        y_sb[:], yps[o][:], ACTF.Identity, bias=bfc_sb[:, o : o + 1]
                    )
                dma_eng = nc.gpsimd if (last and o >= 2) else nc.sync
                dma_eng.dma_start(
                    y_d[o * P : (o + 1) * P, i * tcc : (i + 1) * tcc], y_sb[:]
                )

        # Software-pipelined chunk loop (stages offset on the PE stream).
        # Steady-state iterations interleave L0(i)/L1(i-1) per m-tile so the
        # PE's PSUM-bank production rate tracks the scalar/DVE gate-chain
        # consumption rate instead of bursting ahead during the L0 block.
        def drain(g):
            if g is not None:
                for _ in g:
                    pass

        for i in range(nchunk + 2):
            g0 = layer0_steps(i) if i < nchunk else None
            g1 = layer1_steps(i - 1) if 1 <= i <= nchunk else None
            if i >= 2 and g0 is not None and g1 is not None:
                if i + 1 < nchunk:
                    emit_T(i + 1)
                for _ in range(hsub):
                    next(g0, None)
                    next(g1, None)
                drain(g0)
                drain(g1)
            else:
                drain(g0)
                if i + 1 < nchunk:
                    emit_T(i + 1)
                drain(g1)
            if 2 <= i <= nchunk + 1:
                emit_FC(i - 2)

    nc.compile()
    return nc


_NC_CACHE = {}


def _get_nc(t_total=T, tcc=TC):
    key = (t_total, tcc)
    if key not in _NC_CACHE:
        _NC_CACHE[key] = build(t_total, tcc)
    return _NC_CACHE[key]


_RUNNER = None


def _get_runner():
    """Build (once) a cached jitted SPMD executor for the module so repeated
    kernel() calls reuse the compiled NEFF instead of re-tracing."""
    global _RUNNER
    if _RUNNER is None:
        import jax
        from jax.experimental.shard_map import shard_map
        from jax.sharding import Mesh, PartitionSpec

        from concourse import bass2jax

        bass2jax.install_neuronx_cc_hook()
        nc = _get_nc()
        assert nc.dbg_addr is None
        partition_name = (
            nc.partition_id_tensor.name if nc.partition_id_tensor else None
        )
        in_names, out_names, out_avals = [], [], []
        for alloc in nc.m.functions[0].allocations:
            if not isinstance(alloc, mybir.MemoryLocationSet):
                continue
            name = alloc.memorylocations[0].name
            if alloc.kind == "ExternalInput":
                if name != partition_name:
                    in_names.append(name)
            elif alloc.kind == "ExternalOutput":
                out_names.append(name)
                out_avals.append(
                    jax.core.ShapedArray(
                        tuple(alloc.tensor_shape), mybir.dt.np(alloc.dtype)
                    )
                )
        n_params = len(in_names)
        n_outs = len(out_names)
        all_names = tuple(in_names) + tuple(out_names)
        if partition_name is not None:
            all_names = all_names + (partition_name,)

        def _body(*args):
            operands = list(args)
            if partition_name is not None:
                operands.append(bass2jax.partition_id_tensor())
            outs = bass2jax._bass_exec_p.bind(
                *operands,
                out_avals=tuple(out_avals),
                in_names=all_names,
                out_names=tuple(out_names),
                lowering_input_output_aliases=(),
                sim_require_finite=True,
                sim_require_nnan=True,
                nc=nc,
            )
            return tuple(outs)

        devices = jax.devices()[:B]
        assert len(devices) == B, f"need {B} cores, found {len(jax.devices())}"
        mesh = Mesh(np.asarray(devices), ("core",))
        sharded = jax.jit(
            shard_map(
                _body,
                mesh=mesh,
                in_specs=(PartitionSpec("core"),) * (n_params + n_outs),
                out_specs=(PartitionSpec("core"),) * n_outs,
                check_rep=False,
            ),
            donate_argnums=tuple(range(n_params, n_params + n_outs)),
            keep_unused=True,
        )
        _RUNNER = (sharded, list(in_names), list(out_names), list(out_avals))
    return _RUNNER


def pack_biases(inputs):
    """Host-side bias staging: stripe gate biases (including negated
    z-biases) and the FC bias to [P, 6*hsub + DOUT/P]."""
    hsub = H // P

    def stripe(name):
        a = np.asarray(inputs[name], np.float32)
        return a.reshape(a.size // P, P).T

    pack = np.concatenate(
        [
            stripe("bz0"), stripe("bh0"), stripe("bz1"), stripe("bh1"),
            -stripe("bz0"), -stripe("bz1"), stripe("bfc"),
        ],
        axis=1,
    )
    return {"bias_pack": np.ascontiguousarray(pack)}


def run(inputs, trace=False, **spmd_kwargs):
    """Run the SPMD kernel on all 8 cores. Returns (y[B,T,DOUT], results)."""
    x = np.asarray(inputs["x"], dtype=np.float32)
    assert x.shape == (B, T, DIN), x.shape
    # [B, T, DIN] -> per-core [DIN/P, P, T] bf16 (cast + transpose staging)
    x_bf = np.ascontiguousarray(
        x.astype(ml_dtypes.bfloat16).transpose(0, 2, 1).reshape(B, DIN // P, P, T)
    )
    shared = {}
    for name in ("Wz0", "Wh0", "Wh1", "Wfc"):
        shared[name] = np.ascontiguousarray(
            np.asarray(inputs[name], dtype=np.float32).astype(ml_dtypes.bfloat16)
        )
    # z1 weight: fp8e4 at x16 scale (clipped to the TRN e4m3 finite range).
    shared["Wz1q"] = np.ascontiguousarray(
        np.clip(np.asarray(inputs["Wz1"], np.float32) * WZ1_SCALE, -240.0, 240.0)
        .astype(ml_dtypes.float8_e4m3)
    )
    shared.update(pack_biases(inputs))
    in_maps = [dict(shared, xT=x_bf[c]) for c in range(B)]
    if trace or spmd_kwargs:
        nc = _get_nc()
        res = bass_utils.run_bass_kernel_spmd(
            nc, in_maps, core_ids=list(range(B)), trace=trace, **spmd_kwargs
        )
        # y comes back [DOUT, T] per core; transpose to [T, DOUT].
        y = np.stack(
            [np.asarray(r["y"]).T for r in res.results], axis=0
        ).astype(np.float32)
        return y, res
    sharded, in_names, out_names, out_avals = _get_runner()
    per_core = [[np.asarray(m[n]) for n in in_names] for m in in_maps]
    concat_in = [
        np.concatenate([per_core[c][i] for c in range(B)], axis=0)
        for i in range(len(in_names))
    ]
    concat_zeros = [
        np.zeros((B * a.shape[0], *a.shape[1:]), a.dtype) for a in out_avals
    ]
    outs = sharded(*concat_in, *concat_zeros)
    yi = out_names.index("y")
    y = np.asarray(outs[yi]).reshape(B, *out_avals[yi].shape).astype(np.float32)
    # y is [B, DOUT, T]; transpose to [B, T, DOUT].
    y = np.ascontiguousarray(y.transpose(0, 2, 1))
    return y, None


def kernel(**inputs) -> np.ndarray:
    y, _ = run(inputs)
    return y
